# revision 11
# baseline (speedup 1.0000x reference)
"""MixConv depthwise conv (3x3/5x5/7x7 over 64-channel groups) as banded-Toeplitz
matmuls on the TensorEngine, sharded over 8 NeuronCores by channel.

Decomposition: a kxk depthwise conv = sum over dx of a 1D conv along H applied to
the input shifted by dx along W. The 1D conv along H is a matmul with a banded
[H, H] Toeplitz matrix (built host-side from the conv weights) contracting over
H=112 partitions. W-shifts are free-dim offsets into a padded SBUF image tile;
the dx-passes accumulate in PSUM.

Sharding: 192 channels / 8 cores = 24 channels per core, processed as 8 tiles
of 3 channels (one per kernel-size group) so every tile has identical cost.

Perf structure (verified against HW traces):
- fp16 staging for x and the Toeplitz mats (~4e-4 output rel err), fp16 output
  staging upcast on host: DMA traffic 42MB/core vs 85MB for the fp32 baseline.
- Images are laid out with shared zero gaps of `pad` columns between them
  (stride 112+pad); matmul rhs is a strided [H, 4img, 112] view, so every MM
  streams exactly 448 dense output columns (no junk pad columns in PSUM).
- One LDWEIGHTS per (group, dx) feeds 8 accumulating matmuls across all 8
  PSUM banks (vs per-half reloads: 120 weight loads instead of 960).
- PSUM->SBUF de-pad copies pinned to the Vector engine (ACT copies are ~2x
  slower); Toeplitz mats stay resident in SBUF all kernel.
"""

import numpy as np

import concourse.bacc as bacc
import concourse.mybir as mybir
import concourse.tile as tile
from concourse.bass_utils import run_bass_kernel_spmd

# Problem constants (hardcoded per contract)
N_IMGS = 32
H = W = 112
GROUP_KS = (3, 5, 7)
GROUP_SIZE = 64          # channels per group
N_CORES = 8
N_TILES = 8              # per-core tiles; each holds one channel per group
OC = N_IMGS * W          # 3584 dense output cols per channel

PADS = tuple((k - 1) // 2 for k in GROUP_KS)          # (1, 2, 3)
# images share the inter-image zero gap: region stride = W + pad
STRIDES = tuple(W + p for p in PADS)                  # (113, 114, 115)
# section = pad left cols are implicit in data offset; size covers the last
# chunk's strided-view over-read (base + 4*stride)
SECS = tuple(N_IMGS * s + 2 * p for s, p in zip(STRIDES, PADS))
SEC_OFF = (0, SECS[0], SECS[0] + SECS[1])
XC = sum(SECS)                                        # 10956 fp16 cols per tile
TOFF_G = (0, GROUP_KS[0], GROUP_KS[0] + GROUP_KS[1])  # (0, 3, 8)
N_TMAT_TILE = sum(GROUP_KS)                           # 15 Toeplitz mats per tile
NMM = 4 * W                                           # 448 dense cols per matmul

MM_MODE = "fp16"

_BASS_CACHE = {}


def _build_bass():
    f16 = mybir.dt.float16
    f32 = mybir.dt.float32

    nc = bacc.Bacc("TRN2", target_bir_lowering=False, debug=False)
    xp_d = nc.dram_tensor("xp", [N_TILES, H, XC], f16, kind="ExternalInput")
    t_d = nc.dram_tensor(
        "tmat", [N_TILES, H, N_TMAT_TILE * H], f16, kind="ExternalInput"
    )
    y_d = nc.dram_tensor("y", [N_TILES, 3, H, OC], f16, kind="ExternalOutput")

    # PE needs ~(22.7*ti + 7.5*g) us into the kernel before section (ti, g) is
    # consumed; stagger input-DMA logical priorities (conservatively early) so
    # the first tile's sections aren't queued behind the whole input flood.
    def x_eta_ms(ti, g):
        return max(0.0, (20.0 * ti + 7.0 * g - 12.0) / 1000.0)

    with tile.TileContext(nc) as tc:
        with (
            tc.tile_pool(name="tpool", bufs=N_TILES) as tpool,
            tc.tile_pool(name="xpool", bufs=4) as xpool,
            tc.tile_pool(name="opool", bufs=9) as opool,
            tc.tile_pool(name="pspool", bufs=8, space="PSUM") as pspool,
            tc.tile_pool(name="warmpool", bufs=1) as warmpool,
        ):
            # PE pre-warm: HAM un-throttles after ~3.4us of sustained PE
            # activity; dummy matmuls on a zeroed scratch tile warm the clock
            # gate while the first input DMAs are still landing, so the real
            # matmuls start at 2.4GHz instead of 1.2. The psum tile comes from
            # the main pool and recycles into the pipeline afterwards.
            warm_t = warmpool.tile([H, NMM], f16, tag="warm", name="warm")
            nc.vector.memset(warm_t[:, :], 0.0)
            warm_ps = pspool.tile([H, NMM], f32, tag="ps", name="wps")
            for i in range(18):
                nc.tensor.matmul(
                    warm_ps, lhsT=warm_t[:, :H], rhs=warm_t[:, :], start=True,
                    stop=True,
                )

            t_ts = []
            for ti in range(N_TILES):
                t_t = tpool.tile([H, N_TMAT_TILE * H], f16, tag="t", name=f"t{ti}")
                if ti == 0:
                    # split per group: the first LDWEIGHTS only needs g=0's
                    # 3 matrices, not the whole 0.37MB tile
                    for g, k in enumerate(GROUP_KS):
                        c0, c1 = TOFF_G[g] * H, (TOFF_G[g] + k) * H
                        nc.sync.dma_start(t_t[:, c0:c1], t_d[ti][:, c0:c1])
                else:
                    with tc.tile_wait_until((18.0 * ti - 10.0) / 1000.0):
                        nc.sync.dma_start(t_t[:, :], t_d[ti])
                t_ts.append(t_t)

            for ti in range(N_TILES):
                x_t = xpool.tile([H, XC], f16, tag="x", name=f"x{ti}")
                for g in range(3):
                    with tc.tile_wait_until(x_eta_ms(ti, g)):
                        nc.sync.dma_start(
                            x_t[:, SEC_OFF[g] : SEC_OFF[g] + SECS[g]],
                            xp_d[ti][:, SEC_OFF[g] : SEC_OFF[g] + SECS[g]],
                        )
                for g, k in enumerate(GROUP_KS):
                    s = STRIDES[g]
                    out_t = opool.tile([H, OC], f16, tag="o", name=f"o{ti}_{g}")
                    out_v = out_t.rearrange("p (i w) -> p i w", i=N_IMGS)
                    pts = [
                        pspool.tile([H, NMM], f32, tag="ps", name=f"ps{ti}_{g}_{b}")
                        for b in range(8)
                    ]
                    for dx in range(k):
                        lhsT = t_ts[ti][
                            :, (TOFF_G[g] + dx) * H : (TOFF_G[g] + dx + 1) * H
                        ]
                        for b in range(8):
                            base = SEC_OFF[g] + 4 * b * s + dx
                            rhs = x_t[:, base : base + 4 * s].rearrange(
                                "p (i s) -> p i s", i=4
                            )[:, :, :W]
                            nc.tensor.matmul(
                                pts[b],
                                lhsT=lhsT,
                                rhs=rhs,
                                start=(dx == 0),
                                stop=(dx == k - 1),
                            )
                    last = ti == N_TILES - 1 and g == len(GROUP_KS) - 1
                    for b in range(8):
                        # tail-critical final drain: offload the last two
                        # banks to ACT so the serial DVE chain ends sooner
                        dst = out_v[:, 4 * b : 4 * b + 4, :]
                        src = pts[b].rearrange("p (i w) -> p i w", i=4)
                        if last and b >= 6:
                            nc.scalar.copy(out=dst, in_=src)
                        else:
                            nc.vector.tensor_copy(out=dst, in_=src)
                    if last:
                        # final tile: split the store so it overlaps the last
                        # copy drain instead of serializing after it
                        for q in range(4):
                            c0, c1 = q * OC // 4, (q + 1) * OC // 4
                            nc.sync.dma_start(
                                y_d[ti][g][:, c0:c1], out_t[:, c0:c1]
                            )
                    else:
                        nc.sync.dma_start(y_d[ti][g], out_t[:, :])
    nc.compile()
    return nc


def _get_bass():
    if "v3" not in _BASS_CACHE:
        _BASS_CACHE["v3"] = _build_bass()
    return _BASS_CACHE["v3"]


def _build_toeplitz(w, k):
    """w: [C, 1, k, k] -> T: [C, k, H, H], T[c,dx,hin,hout] = w[c,0,hin-hout+pad,dx]."""
    pad = (k - 1) // 2
    C = w.shape[0]
    T = np.zeros((C, k, H, H), np.float32)
    for dy in range(k):
        off = pad - dy  # hout = hin + off
        hin = np.arange(max(0, -off), H - max(0, off))
        T[:, :, hin, hin + off] = w[:, 0, dy, :][:, :, None]
    return T


def _prepare_in_maps(x, w3, w5, w7):
    x = np.ascontiguousarray(np.asarray(x, dtype=np.float32))
    ws = {3: np.asarray(w3, np.float32), 5: np.asarray(w5, np.float32),
          7: np.asarray(w7, np.float32)}
    Ts = {k: _build_toeplitz(ws[k], k) for k in GROUP_KS}

    in_maps = []
    for core in range(N_CORES):
        xp = np.zeros((N_TILES, H, XC), np.float16)
        tm = np.zeros((N_TILES, H, N_TMAT_TILE * H), np.float16)
        for g, k in enumerate(GROUP_KS):
            pad, s = PADS[g], STRIDES[g]
            gchs = [core * N_TILES + ti for ti in range(N_TILES)]  # within group
            chs = [GROUP_SIZE * g + c for c in gchs]               # global
            sec = xp[:, :, SEC_OFF[g] : SEC_OFF[g] + N_IMGS * s].reshape(
                N_TILES, H, N_IMGS, s
            )
            # x[img, ch, h, w] -> sec[ti, h, img, pad+w]; gaps stay zero
            sec[:, :, :, pad : pad + W] = x[:, chs].transpose(1, 2, 0, 3)
            # Toeplitz [ch, dx, hin, hout] -> tm[ti, hin, (TOFF+dx)*H + hout]
            tmg = Ts[k][gchs].transpose(0, 2, 1, 3).reshape(N_TILES, H, k * H)
            tm[:, :, TOFF_G[g] * H : (TOFF_G[g] + k) * H] = tmg
        in_maps.append({"xp": xp, "tmat": tm})
    return in_maps


def _gather(results):
    out = np.empty((N_IMGS, GROUP_SIZE * len(GROUP_KS), H, W), np.float32)
    for core in range(N_CORES):
        y = results[core]["y"].astype(np.float32).reshape(N_TILES, 3, H, N_IMGS, W)
        for g in range(len(GROUP_KS)):
            chs = [GROUP_SIZE * g + core * N_TILES + ti for ti in range(N_TILES)]
            # y[ti, g, h, img, w] -> out[img, ch, h, w]
            out[:, chs] = y[:, g].transpose(2, 0, 1, 3)
    return out


def run(x, w3, w5, w7, **spmd_kwargs):
    """Full run; returns (output, BassKernelResults) for profiling access."""
    nc = _get_bass()
    in_maps = _prepare_in_maps(x, w3, w5, w7)
    br = run_bass_kernel_spmd(nc, in_maps, core_ids=list(range(N_CORES)), **spmd_kwargs)
    return _gather(br.results), br


def kernel(x, w3, w5, w7):
    out, _ = run(x, w3, w5, w7)
    return out
